# revision 2
# baseline (speedup 1.0000x reference)
"""Bass/Tile HBV kernel for 8 TRN2 NeuronCores.

Bulk reformulation: per chunk of 128 cells (partition dim) x 730 days (free dim),
the HBV recurrences become hardware tensor_tensor_scan instructions plus bulk
elementwise ops; nonlinear buckets are solved by short Picard/Newton iterations
(validated in numpy to converge well below the 2e-2 gate).

Cells are ranked by a cheap host-side coarse simulation of the SUZ regime
iteration's convergence and striped across cores so each chunk-slot is
difficulty-homogeneous; harder slots run more SUZ regime iterations. Routing
convolution runs in fp16 (validated).

Self-contained: needs numpy + concourse (+ axon TRN2 devices).
"""
import numpy as np

import concourse.bacc as bacc
import concourse.mybir as mybir
from concourse.bass_utils import run_bass_kernel_spmd
from concourse.tile import TileContext

F32 = mybir.dt.float32
F16 = mybir.dt.float16
AL = mybir.AluOpType

T = 730
G_FULL = 10000
N_CORES = 8
CHUNKS_PER_CORE = 10
P128 = 128
G_PAD = N_CORES * CHUNKS_PER_CORE * P128  # 10240
LENF = 15
NEARZERO = 1e-5

PHY_BOUNDS = [
    ("parBETA", 1.0, 6.0), ("parFC", 50.0, 1000.0), ("parK0", 0.05, 0.9),
    ("parK1", 0.01, 0.5), ("parK2", 0.001, 0.2), ("parLP", 0.2, 1.0),
    ("parPERC", 0.0, 10.0), ("parUZL", 0.0, 100.0), ("parTT", -2.5, 2.5),
    ("parCFMAX", 0.5, 10.0), ("parCFR", 0.0, 0.1), ("parCWH", 0.0, 0.2),
]
ROUT_A_BOUNDS = (0.0, 2.9)
ROUT_B_BOUNDS = (0.0, 6.5)

# const column indices
(C_TT, C_MS, C_MB, C_RS, C_RB, C_1CWH, C_FC, C_BETA, C_BLIF, C_BM1, C_SWPB,
 C_ILPFC, C_PCAP, C_PCUZ, C_CA, C_CB, C_C3, C_1K2, C_KAP, C_FCH) = range(20)
C_W0 = 20
NCONST = C_W0 + LENF  # 35

# iteration counts; slot 0 = hardest cells (per host difficulty ranking)
N_A = 2
N_B = 4
SLOT_NS = (18, 14, 10, 8, 6, 5, 5, 5, 5, 5)


def _sigmoid(x):
    return 1.0 / (1.0 + np.exp(-x))


def host_params(par_last):
    phy = _sigmoid(par_last[:, :12].astype(np.float64))
    rout = _sigmoid(par_last[:, 12:].astype(np.float64))
    p = {}
    for i, (nm, lo, hi) in enumerate(PHY_BOUNDS):
        p[nm] = lo + phy[:, i] * (hi - lo)
    p["rout_a"] = ROUT_A_BOUNDS[0] + rout[:, 0] * (ROUT_A_BOUNDS[1] - ROUT_A_BOUNDS[0])
    p["rout_b"] = ROUT_B_BOUNDS[0] + rout[:, 1] * (ROUT_B_BOUNDS[1] - ROUT_B_BOUNDS[0])
    return p


def host_consts(p):
    g = len(p["parTT"])
    c = np.zeros((g, NCONST), np.float64)
    TTp = p["parTT"]; CFMAX = p["parCFMAX"]; CFR = p["parCFR"]
    c[:, C_TT] = TTp
    c[:, C_MS] = CFMAX
    c[:, C_MB] = -CFMAX * TTp
    c[:, C_RS] = -CFR * CFMAX
    c[:, C_RB] = CFR * CFMAX * TTp
    c[:, C_1CWH] = 1.0 + p["parCWH"]
    c[:, C_FC] = p["parFC"]
    c[:, C_BETA] = p["parBETA"]
    lnInvFC = -np.log(p["parFC"])
    c[:, C_BLIF] = p["parBETA"] * lnInvFC
    c[:, C_BM1] = p["parBETA"] - 1.0
    c[:, C_SWPB] = p["parBETA"] * lnInvFC + np.log(p["parBETA"])
    c[:, C_ILPFC] = 1.0 / (p["parLP"] * p["parFC"])
    c[:, C_PCAP] = p["parPERC"]
    c[:, C_PCUZ] = p["parPERC"] + p["parUZL"]
    ca = 1.0 - p["parK1"]
    c[:, C_CA] = ca
    c[:, C_CB] = -p["parK0"] * ca
    c[:, C_C3] = ca * p["parK0"] * p["parUZL"]
    c[:, C_1K2] = 1.0 - p["parK2"]
    c[:, C_KAP] = p["parK2"] / (1.0 - p["parK2"])
    c[:, C_FCH] = 0.5 * p["parFC"]
    aa = np.maximum(p["rout_a"], 0.0) + 0.1
    theta = np.maximum(p["rout_b"], 0.0) + 0.5
    tk = np.arange(LENF, dtype=np.float64) + 0.5
    wv = np.exp((aa[:, None] - 1.0) * np.log(tk)[None, :]
                - tk[None, :] / theta[:, None])
    c[:, C_W0:C_W0 + LENF] = wv / wv.sum(axis=1, keepdims=True)
    return c.astype(np.float32)


def difficulty(p, x_phy, stride=4, k_lo=4, k_hi=9):
    """Per-cell SUZ iteration difficulty: residual between k_lo and k_hi regime
    iterations of a coarse (time-strided) SUZ solve with a proxy inflow."""
    P = x_phy[::stride, :, 0].astype(np.float64)
    PET = x_phy[::stride, :, 2].astype(np.float64)
    SUZIN = np.maximum(P - 0.7 * PET, 0.0)
    Tc, G = SUZIN.shape
    K0 = p["parK0"]; K1 = p["parK1"]; PCAP = p["parPERC"]; UZL = p["parUZL"]
    ca = 1.0 - K1
    SUZ_prev = np.zeros((Tc, G))
    keep = {}
    SUZ = np.zeros((Tc, G))
    for it in range(k_hi):
        S1 = SUZ_prev + SUZIN
        m1 = S1 > PCAP
        m2 = S1 > PCAP + UZL
        alpha = ca * (1.0 - K0 * m2) * m1
        beta = alpha * (SUZIN - PCAP) + (ca * K0 * UZL) * m2
        s = np.zeros(G)
        for t in range(Tc):
            s = alpha[t] * s + beta[t]
            SUZ[t] = s
        if it + 1 in (k_lo, k_hi):
            keep[it + 1] = SUZ.copy()
        SUZ_prev[1:] = SUZ[:-1]
        SUZ_prev[0] = 0.0
    return np.abs(keep[k_hi] - keep[k_lo]).mean(axis=0)


def build_nc(n_a=N_A, n_b=N_B, slot_ns=SLOT_NS):
    nc = bacc.Bacc("TRN2", target_bir_lowering=False, debug=False,
                   num_devices=N_CORES)
    din = {}
    for nm in ("pp", "tm", "pe"):
        din[nm] = nc.declare_dram_parameter(nm, [CHUNKS_PER_CORE, P128, T], F32,
                                            isOutput=False)
    din["cc"] = nc.declare_dram_parameter("cc", [CHUNKS_PER_CORE, P128, NCONST],
                                          F32, isOutput=False)
    dout = nc.declare_dram_parameter("y", [CHUNKS_PER_CORE, P128, T], F32,
                                     isOutput=True)
    with TileContext(nc) as tc:
        with tc.tile_pool(name="gl", bufs=1) as gpool:
            zeros = gpool.tile([P128, T], F32, name="zeros")
            nc.vector.memset(zeros[:, :], 0.0)
            ones = gpool.tile([P128, T], F32, name="ones")
            nc.vector.memset(ones[:, :], 1.0)
            with tc.tile_pool(name="io", bufs=2) as iop, \
                    tc.tile_pool(name="wk", bufs=2) as wk:
                for ci in range(CHUNKS_PER_CORE):
                    _chunk(nc, (iop, wk), din, dout, ci, zeros, ones,
                           n_a, n_b, slot_ns[ci])
    nc.compile()
    return nc


def _chunk(nc, pools, din, dout, ci, zeros, ones, n_a, n_b, n_s):
    iop, wk = pools
    V = nc.vector
    A = nc.scalar
    AF = mybir.ActivationFunctionType
    dma = nc.sync.dma_start

    def tl(tag, w=T, dt=F32):
        return wk.tile([P128, w], dt, tag=tag, name=tag)

    # io planes
    Pp = iop.tile([P128, T], F32, tag="Pp", name="Pp")
    TMp = iop.tile([P128, T], F32, tag="TMp", name="TMp")
    PEp = iop.tile([P128, T], F32, tag="PEp", name="PEp")
    ct = iop.tile([P128, NCONST], F32, tag="ct", name="ct")
    dma(Pp[:, :], din["pp"][ci])
    dma(TMp[:, :], din["tm"][ci])
    dma(PEp[:, :], din["pe"][ci])
    dma(ct[:, :], din["cc"][ci])

    def cc(i):
        return ct[:, i:i + 1]

    # scratch planes (explicitly reused across phases)
    s0 = tl("s0"); s1 = tl("s1"); s2 = tl("s2"); s3 = tl("s3")
    s4 = tl("s4"); s5 = tl("s5"); s6 = tl("s6")

    # ---- stage 0 ----
    SNOW = tl("SNOW"); Aa = tl("Aa"); negR = tl("negR"); PETinv = tl("PETinv")
    A.activation(s0[:, :], TMp[:, :], AF.Relu, scale=cc(C_MS), bias=cc(C_MB))  # M
    A.activation(s1[:, :], TMp[:, :], AF.Relu, scale=cc(C_RS), bias=cc(C_RB))  # R
    A.activation(negR[:, :], s1[:, :], AF.Copy, scale=-1.0)
    V.tensor_scalar(s2[:, :], TMp[:, :], cc(C_TT), None, AL.is_lt)
    V.tensor_tensor(SNOW[:, :], Pp[:, :], s2[:, :], AL.mult)
    V.tensor_tensor(Aa[:, :], SNOW[:, :], s0[:, :], AL.subtract)
    A.activation(PETinv[:, :], PEp[:, :], AF.Copy, scale=cc(C_ILPFC))

    # ---- snow ----
    Xb = tl("Xb"); Wb = tl("Wb", T + 1); cbuf = tl("cbuf", T + 1)
    negMW = tl("negMW", T + 1)
    V.memset(Wb[:, 0:1], 0.002)
    V.memset(cbuf[:, 0:1], 0.0)
    V.memset(negMW[:, 0:1], -0.001)
    sp = None
    for it in range(n_a):
        if it == 0:
            V.tensor_tensor_scan(Xb[:, :], Aa[:, :], zeros[:, :], 0.001,
                                 AL.add, AL.max)
            sp = Xb
        else:
            V.tensor_tensor(negMW[:, 1:T + 1], sp[:, :], Wb[:, 1:T + 1],
                            AL.subtract)
            V.scalar_tensor_tensor(s0[:, :], negMW[:, 0:T], 0.0, negR[:, :],
                                   AL.min, AL.max)                       # -r
            V.tensor_tensor_scan(cbuf[:, 1:T + 1], s0[:, :], s0[:, :], 0.0,
                                 AL.add, AL.bypass)                      # -cumsum r
            V.tensor_tensor_scan(Xb[:, :], Aa[:, :], cbuf[:, 0:T], 0.001,
                                 AL.add, AL.max)
            V.tensor_tensor(s1[:, :], Xb[:, :], cbuf[:, 1:T + 1], AL.subtract)
            sp = s1
        A.activation(s2[:, :], sp[:, :], AF.Copy, scale=cc(C_1CWH))
        V.tensor_tensor_scan(Wb[:, 1:T + 1], SNOW[:, :], s2[:, :], 0.002,
                             AL.add, AL.min)
    INb = tl("INb")
    V.tensor_tensor(s0[:, :], Wb[:, 0:T], Wb[:, 1:T + 1], AL.subtract)
    V.tensor_tensor(INb[:, :], s0[:, :], Pp[:, :], AL.add)

    # ---- soil: clamped Newton; reuse last iteration's e for SUZIN ----
    SMb = tl("SMb", T + 1); eb = tl("eb")
    V.memset(SMb[:, 0:1], 0.001)
    A.activation(SMb[:, 1:T + 1], ones[:, :], AF.Copy, scale=cc(C_FCH))
    for it in range(n_b):
        Sprev = SMb[:, 0:T]
        A.activation(s0[:, :], Sprev, AF.Ln)
        A.activation(s1[:, :], s0[:, :], AF.Exp, scale=cc(C_BETA), bias=cc(C_BLIF))  # sw
        A.activation(s2[:, :], s0[:, :], AF.Exp, scale=cc(C_BM1), bias=cc(C_SWPB))   # swp
        A.activation(s0[:, :], s1[:, :], AF.Copy, scale=-1.0, bias=1.0)   # 1-sw
        V.tensor_tensor(s1[:, :], s0[:, :], INb[:, :], AL.mult)           # u
        V.tensor_tensor(s3[:, :], SMb[:, 0:T], s1[:, :], AL.add)          # SMa
        V.tensor_scalar(s4[:, :], s3[:, :], cc(C_FC), None, AL.min)       # SMmid
        V.tensor_scalar(s5[:, :], s4[:, :], cc(C_ILPFC), 1.0, AL.mult, AL.min)  # q
        V.tensor_tensor(eb[:, :], PEp[:, :], s5[:, :], AL.mult)           # e
        V.tensor_tensor(s6[:, :], s4[:, :], eb[:, :], AL.subtract)        # fval
        V.tensor_scalar(s0[:, :], s3[:, :], cc(C_FC), None, AL.is_lt)     # mFC
        V.tensor_tensor(s1[:, :], INb[:, :], s2[:, :], AL.mult)
        A.activation(s1[:, :], s1[:, :], AF.Copy, scale=-1.0, bias=1.0)   # 1-IN*swp
        V.tensor_scalar(s2[:, :], s5[:, :], 1.0, None, AL.is_lt)          # mEF
        V.tensor_tensor(s2[:, :], s2[:, :], PETinv[:, :], AL.mult)
        A.activation(s2[:, :], s2[:, :], AF.Copy, scale=-1.0, bias=1.0)
        V.tensor_tensor(s3[:, :], s0[:, :], s1[:, :], AL.mult)
        V.tensor_tensor(s4[:, :], s3[:, :], s2[:, :], AL.mult)
        V.tensor_scalar(s5[:, :], s4[:, :], 0.0, 1.0, AL.max, AL.min)     # a
        V.tensor_tensor(s0[:, :], s6[:, :], SMb[:, 1:T + 1], AL.subtract)  # rho
        V.tensor_tensor_scan(s1[:, :], s5[:, :], s0[:, :], 0.0, AL.mult, AL.add)
        V.tensor_tensor(s2[:, :], SMb[:, 1:T + 1], s1[:, :], AL.add)
        V.tensor_scalar(s3[:, :], s2[:, :], NEARZERO, None, AL.max)
        V.tensor_scalar(SMb[:, 1:T + 1], s3[:, :], cc(C_FC), None, AL.min)
    SUZIN = tl("SUZIN")
    V.tensor_tensor(s0[:, :], INb[:, :], eb[:, :], AL.subtract)
    V.tensor_tensor(s1[:, :], SMb[:, 1:T + 1], SMb[:, 0:T], AL.subtract)
    V.tensor_tensor(SUZIN[:, :], s0[:, :], s1[:, :], AL.subtract)

    # ---- SUZ regime iteration (early iterations fp16, last 2 fp32) ----
    SUZb = tl("SUZb", T + 1); SINP = tl("SINP")
    SUZh = tl("SUZh", T + 1, F16)
    SUZINh = tl("SUZINh", T, F16); SINPh = tl("SINPh", T, F16)
    h0 = tl("h0", T, F16); h1 = tl("h1", T, F16); h2 = tl("h2", T, F16)
    h3 = tl("h3", T, F16); h4 = tl("h4", T, F16); h5 = tl("h5", T, F16)
    V.memset(SUZb[:, 0:1], 0.001)
    V.memset(SUZh[:, 0:1], 0.001)
    V.memset(SUZh[:, 1:T + 1], 0.001)
    V.tensor_scalar(SINP[:, :], SUZIN[:, :], cc(C_PCAP), None, AL.subtract)
    V.tensor_copy(SUZINh[:, :], SUZIN[:, :])
    V.tensor_copy(SINPh[:, :], SINP[:, :])
    for it in range(n_s):
        if it < n_s - 2:
            V.tensor_tensor(h0[:, :], SUZh[:, 0:T], SUZINh[:, :], AL.add)   # S1
            V.tensor_scalar(h1[:, :], h0[:, :], cc(C_PCAP), None, AL.is_gt)
            V.tensor_scalar(h2[:, :], h0[:, :], cc(C_PCUZ), None, AL.is_gt)
            V.tensor_scalar(h3[:, :], h1[:, :], cc(C_CA), None, AL.mult)
            V.scalar_tensor_tensor(h4[:, :], h2[:, :], cc(C_CB), h3[:, :],
                                   AL.mult, AL.add)                         # alpha
            V.tensor_tensor(h5[:, :], h4[:, :], SINPh[:, :], AL.mult)
            V.scalar_tensor_tensor(h5[:, :], h2[:, :], cc(C_C3), h5[:, :],
                                   AL.mult, AL.add)                         # beta
            V.tensor_tensor_scan(SUZh[:, 1:T + 1], h4[:, :], h5[:, :], 0.001,
                                 AL.mult, AL.add)
            continue
        prevb = SUZh if it == n_s - 2 else SUZb
        V.tensor_tensor(s0[:, :], prevb[:, 0:T], SUZIN[:, :], AL.add)      # S1
        V.tensor_scalar(s1[:, :], s0[:, :], cc(C_PCAP), None, AL.is_gt)    # m1
        V.tensor_scalar(s2[:, :], s0[:, :], cc(C_PCUZ), None, AL.is_gt)    # m2
        V.tensor_scalar(s3[:, :], s1[:, :], cc(C_CA), None, AL.mult)
        V.scalar_tensor_tensor(s4[:, :], s2[:, :], cc(C_CB), s3[:, :],
                               AL.mult, AL.add)                            # alpha
        V.tensor_tensor(s5[:, :], s4[:, :], SINP[:, :], AL.mult)
        V.scalar_tensor_tensor(s6[:, :], s2[:, :], cc(C_C3), s5[:, :],
                               AL.mult, AL.add)                            # beta
        V.tensor_tensor_scan(SUZb[:, 1:T + 1], s4[:, :], s6[:, :], 0.001,
                             AL.mult, AL.add)
    V.tensor_tensor(s0[:, :], SUZb[:, 0:T], SUZIN[:, :], AL.add)           # S1
    V.tensor_scalar(s1[:, :], s0[:, :], cc(C_PCAP), None, AL.min)          # PERC
    V.tensor_tensor(s2[:, :], s0[:, :], s1[:, :], AL.subtract)
    V.tensor_tensor(s3[:, :], s2[:, :], SUZb[:, 1:T + 1], AL.subtract)     # Q01

    # ---- SLZ ----
    A.activation(s4[:, :], ones[:, :], AF.Copy, scale=cc(C_1K2))
    A.activation(s5[:, :], s1[:, :], AF.Copy, scale=cc(C_1K2))
    V.tensor_tensor_scan(s6[:, :], s4[:, :], s5[:, :], 0.001, AL.mult, AL.add)  # SLZ
    A.activation(s0[:, :], s6[:, :], AF.Copy, scale=cc(C_KAP))             # Q2

    # ---- routing conv in fp16 ----
    QbH = tl("QbH", T + LENF - 1, F16)
    yA = tl("yA", T, F16)
    yB = tl("yB", T, F16)
    V.memset(QbH[:, 0:LENF - 1], 0.0)
    V.tensor_tensor(QbH[:, LENF - 1:T + LENF - 1], s3[:, :], s0[:, :], AL.add)  # Q
    wq = tl("wq", T, F16)
    base = LENF - 1
    V.tensor_scalar(yA[:, :], QbH[:, base:base + T], cc(C_W0), None, AL.mult)
    src, dst = yA, yB
    for k in range(1, LENF):
        V.tensor_scalar(wq[:, :], QbH[:, base - k:base - k + T],
                        cc(C_W0 + k), None, AL.mult)
        V.tensor_tensor(dst[:, :], src[:, :], wq[:, :], AL.add)
        src, dst = dst, src
    nc.gpsimd.dma_start(dout[ci], src[:, :])  # casts fp16 -> fp32


# ---------------- host orchestration ----------------
_CACHE = {}


def _get_nc(key=None):
    if key is None:
        key = (N_A, N_B, SLOT_NS)
    if key not in _CACHE:
        _CACHE[key] = build_nc(*key)
    return _CACHE[key]


def cell_layout(p, x_phy):
    """Position i (0..G_PAD-1) holds source cell cells[i]; chunk g=i//128 maps
    to core g%8, slot g//8. Hardest cells first so low slots are hard.
    Difficulty = union of two rankings: coarse-sim SUZ residual and the
    persistence heuristic (a cell is hard if either says so)."""
    G = G_FULL
    dsim = difficulty(p, x_phy)
    Pm = x_phy[:, :, 0].mean(axis=0).astype(np.float64)
    PETm = x_phy[:, :, 2].mean(axis=0).astype(np.float64)
    dcrude = (1.0 - p["parK1"]) * (Pm - 0.7 * PETm > p["parPERC"])

    def ranks(d):
        o = np.argsort(-d, kind="stable")
        r = np.empty(G, np.int64)
        r[o] = np.arange(G)
        return r

    runion = np.minimum(ranks(dsim), ranks(dcrude))
    pad = np.arange(G_PAD - G)
    rall = np.concatenate([runion, runion[pad]])
    order = np.argsort(rall, kind="stable")
    cells = np.concatenate([np.arange(G), pad])[order]
    gchunk = np.arange(G_PAD) // P128
    core_of = gchunk % N_CORES
    return cells, core_of


def kernel(x_phy: np.ndarray, parameters: np.ndarray, trace=False):
    x = np.asarray(x_phy, np.float32)
    par_last = np.asarray(parameters)[-1].astype(np.float32)
    Tn, G, _ = x.shape
    assert Tn == T and G == G_FULL
    p = host_params(par_last)
    cells, core_of = cell_layout(p, x)
    consts_all = host_consts(p)[cells]
    xg = x[:, cells, :]
    in_maps = []
    per_core = CHUNKS_PER_CORE * P128
    for c in range(N_CORES):
        idx = np.where(core_of == c)[0]
        blk = np.ascontiguousarray(np.moveaxis(xg[:, idx, :], 0, 1))  # [pc, T, 3]
        in_maps.append({
            "pp": np.ascontiguousarray(blk[:, :, 0]).reshape(CHUNKS_PER_CORE, P128, T),
            "tm": np.ascontiguousarray(blk[:, :, 1]).reshape(CHUNKS_PER_CORE, P128, T),
            "pe": np.ascontiguousarray(blk[:, :, 2]).reshape(CHUNKS_PER_CORE, P128, T),
            "cc": np.ascontiguousarray(consts_all[idx]).reshape(CHUNKS_PER_CORE, P128, NCONST),
        })
    nc = _get_nc()
    res = run_bass_kernel_spmd(nc, in_maps, list(range(N_CORES)), trace=trace)
    out = np.empty((T, G), np.float32)
    for c in range(N_CORES):
        idx = np.where(core_of == c)[0]
        ys = res.results[c]["y"].reshape(per_core, T)
        out[:, cells[idx]] = ys.T  # pad duplicates overwrite identically
    if trace:
        return out, res
    return out


# revision 4
# speedup vs baseline: 1.0303x; 1.0303x over previous
"""Bass/Tile HBV kernel for 8 TRN2 NeuronCores.

Bulk reformulation: per chunk of 128 cells (partition dim) x 730 days (free dim),
the HBV recurrences become hardware tensor_tensor_scan instructions plus bulk
elementwise ops; nonlinear buckets are solved by short Picard/Newton iterations
(validated in numpy to converge well below the 2e-2 gate).

Cells are ranked by a cheap host-side coarse simulation of the SUZ regime
iteration's convergence and striped across cores so each chunk-slot is
difficulty-homogeneous; harder slots run more SUZ regime iterations. Routing
convolution runs in fp16 (validated).

Self-contained: needs numpy + concourse (+ axon TRN2 devices).
"""
import numpy as np

import concourse.bacc as bacc
import concourse.mybir as mybir
from concourse.bass_utils import run_bass_kernel_spmd
from concourse.tile import TileContext

F32 = mybir.dt.float32
F16 = mybir.dt.float16
AL = mybir.AluOpType

T = 730
G_FULL = 10000
N_CORES = 8
CHUNKS_PER_CORE = 10
P128 = 128
G_PAD = N_CORES * CHUNKS_PER_CORE * P128  # 10240
LENF = 15
NEARZERO = 1e-5

PHY_BOUNDS = [
    ("parBETA", 1.0, 6.0), ("parFC", 50.0, 1000.0), ("parK0", 0.05, 0.9),
    ("parK1", 0.01, 0.5), ("parK2", 0.001, 0.2), ("parLP", 0.2, 1.0),
    ("parPERC", 0.0, 10.0), ("parUZL", 0.0, 100.0), ("parTT", -2.5, 2.5),
    ("parCFMAX", 0.5, 10.0), ("parCFR", 0.0, 0.1), ("parCWH", 0.0, 0.2),
]
ROUT_A_BOUNDS = (0.0, 2.9)
ROUT_B_BOUNDS = (0.0, 6.5)

# const column indices
(C_TT, C_MS, C_MB, C_RS, C_RB, C_1CWH, C_FC, C_BETA, C_BLIF, C_BM1, C_SWPB,
 C_ILPFC, C_PCAP, C_PCUZ, C_CA, C_CB, C_C3, C_1K2, C_KAP, C_FCH) = range(20)
C_W0 = 20
NCONST = C_W0 + LENF  # 35

# iteration counts; slot 0 = hardest cells (per host difficulty ranking)
N_A = 2
N_B = 4
SLOT_NS = (18, 14, 10, 8, 6, 5, 5, 5, 5, 5)


def _sigmoid(x):
    return 1.0 / (1.0 + np.exp(-x))


def host_params(par_last):
    phy = _sigmoid(par_last[:, :12].astype(np.float64))
    rout = _sigmoid(par_last[:, 12:].astype(np.float64))
    p = {}
    for i, (nm, lo, hi) in enumerate(PHY_BOUNDS):
        p[nm] = lo + phy[:, i] * (hi - lo)
    p["rout_a"] = ROUT_A_BOUNDS[0] + rout[:, 0] * (ROUT_A_BOUNDS[1] - ROUT_A_BOUNDS[0])
    p["rout_b"] = ROUT_B_BOUNDS[0] + rout[:, 1] * (ROUT_B_BOUNDS[1] - ROUT_B_BOUNDS[0])
    return p


def host_consts(p):
    g = len(p["parTT"])
    c = np.zeros((g, NCONST), np.float64)
    TTp = p["parTT"]; CFMAX = p["parCFMAX"]; CFR = p["parCFR"]
    c[:, C_TT] = TTp
    c[:, C_MS] = CFMAX
    c[:, C_MB] = -CFMAX * TTp
    c[:, C_RS] = -CFR * CFMAX
    c[:, C_RB] = CFR * CFMAX * TTp
    c[:, C_1CWH] = 1.0 + p["parCWH"]
    c[:, C_FC] = p["parFC"]
    c[:, C_BETA] = p["parBETA"]
    lnInvFC = -np.log(p["parFC"])
    c[:, C_BLIF] = p["parBETA"] * lnInvFC
    c[:, C_BM1] = p["parBETA"] - 1.0
    c[:, C_SWPB] = p["parBETA"] * lnInvFC + np.log(p["parBETA"])
    c[:, C_ILPFC] = 1.0 / (p["parLP"] * p["parFC"])
    c[:, C_PCAP] = p["parPERC"]
    c[:, C_PCUZ] = p["parPERC"] + p["parUZL"]
    ca = 1.0 - p["parK1"]
    c[:, C_CA] = ca
    c[:, C_CB] = -p["parK0"] * ca
    c[:, C_C3] = ca * p["parK0"] * p["parUZL"]
    c[:, C_1K2] = 1.0 - p["parK2"]
    c[:, C_KAP] = p["parK2"] / (1.0 - p["parK2"])
    c[:, C_FCH] = 0.5 * p["parFC"]
    aa = np.maximum(p["rout_a"], 0.0) + 0.1
    theta = np.maximum(p["rout_b"], 0.0) + 0.5
    tk = np.arange(LENF, dtype=np.float64) + 0.5
    wv = np.exp((aa[:, None] - 1.0) * np.log(tk)[None, :]
                - tk[None, :] / theta[:, None])
    c[:, C_W0:C_W0 + LENF] = wv / wv.sum(axis=1, keepdims=True)
    return c.astype(np.float32)


def difficulty(p, x_phy, stride=4, k_lo=4, k_hi=9):
    """Per-cell SUZ iteration difficulty: residual between k_lo and k_hi regime
    iterations of a coarse (time-strided) SUZ solve with a proxy inflow."""
    P = x_phy[::stride, :, 0].astype(np.float64)
    PET = x_phy[::stride, :, 2].astype(np.float64)
    SUZIN = np.maximum(P - 0.7 * PET, 0.0)
    Tc, G = SUZIN.shape
    K0 = p["parK0"]; K1 = p["parK1"]; PCAP = p["parPERC"]; UZL = p["parUZL"]
    ca = 1.0 - K1
    SUZ_prev = np.zeros((Tc, G))
    keep = {}
    SUZ = np.zeros((Tc, G))
    for it in range(k_hi):
        S1 = SUZ_prev + SUZIN
        m1 = S1 > PCAP
        m2 = S1 > PCAP + UZL
        alpha = ca * (1.0 - K0 * m2) * m1
        beta = alpha * (SUZIN - PCAP) + (ca * K0 * UZL) * m2
        s = np.zeros(G)
        for t in range(Tc):
            s = alpha[t] * s + beta[t]
            SUZ[t] = s
        if it + 1 in (k_lo, k_hi):
            keep[it + 1] = SUZ.copy()
        SUZ_prev[1:] = SUZ[:-1]
        SUZ_prev[0] = 0.0
    return np.abs(keep[k_hi] - keep[k_lo]).mean(axis=0)


def build_nc(n_a=N_A, n_b=N_B, slot_ns=SLOT_NS):
    nc = bacc.Bacc("TRN2", target_bir_lowering=False, debug=False,
                   num_devices=N_CORES)
    din = {}
    for nm in ("pp", "tm", "pe"):
        din[nm] = nc.declare_dram_parameter(nm, [CHUNKS_PER_CORE, P128, T], F32,
                                            isOutput=False)
    din["cc"] = nc.declare_dram_parameter("cc", [CHUNKS_PER_CORE, P128, NCONST],
                                          F32, isOutput=False)
    dout = nc.declare_dram_parameter("y", [CHUNKS_PER_CORE, P128, T], F32,
                                     isOutput=True)
    with TileContext(nc) as tc:
        with tc.tile_pool(name="gl", bufs=1) as gpool:
            zeros = gpool.tile([P128, T], F32, name="zeros")
            nc.vector.memset(zeros[:, :], 0.0)
            ones = gpool.tile([P128, T], F32, name="ones")
            nc.vector.memset(ones[:, :], 1.0)
            with tc.tile_pool(name="io", bufs=2) as iop, \
                    tc.tile_pool(name="wk", bufs=2) as wk:
                for ci in range(CHUNKS_PER_CORE):
                    _chunk(nc, (iop, wk), din, dout, ci, zeros, ones,
                           n_a, n_b, slot_ns[ci])
    nc.compile()
    return nc


def _chunk(nc, pools, din, dout, ci, zeros, ones, n_a, n_b, n_s):
    iop, wk = pools
    V = nc.vector
    A = nc.scalar
    AF = mybir.ActivationFunctionType
    dma = nc.sync.dma_start

    def tl(tag, w=T, dt=F32):
        return wk.tile([P128, w], dt, tag=tag, name=tag)

    # io planes
    Pp = iop.tile([P128, T], F32, tag="Pp", name="Pp")
    TMp = iop.tile([P128, T], F32, tag="TMp", name="TMp")
    PEp = iop.tile([P128, T], F32, tag="PEp", name="PEp")
    ct = iop.tile([P128, NCONST], F32, tag="ct", name="ct")
    dma(Pp[:, :], din["pp"][ci])
    dma(TMp[:, :], din["tm"][ci])
    dma(PEp[:, :], din["pe"][ci])
    dma(ct[:, :], din["cc"][ci])

    def cc(i):
        return ct[:, i:i + 1]

    # scratch planes (explicitly reused across phases)
    s0 = tl("s0"); s1 = tl("s1"); s2 = tl("s2"); s3 = tl("s3")
    s4 = tl("s4"); s5 = tl("s5"); s6 = tl("s6")

    # ---- stage 0 ----
    SNOW = tl("SNOW"); Aa = tl("Aa"); negR = tl("negR"); PETinv = tl("PETinv")
    A.activation(s0[:, :], TMp[:, :], AF.Relu, scale=cc(C_MS), bias=cc(C_MB))  # M
    A.activation(s1[:, :], TMp[:, :], AF.Relu, scale=cc(C_RS), bias=cc(C_RB))  # R
    A.activation(negR[:, :], s1[:, :], AF.Copy, scale=-1.0)
    V.tensor_scalar(s2[:, :], TMp[:, :], cc(C_TT), None, AL.is_lt)
    V.tensor_tensor(SNOW[:, :], Pp[:, :], s2[:, :], AL.mult)
    V.tensor_tensor(Aa[:, :], SNOW[:, :], s0[:, :], AL.subtract)
    A.activation(PETinv[:, :], PEp[:, :], AF.Copy, scale=cc(C_ILPFC))

    # ---- snow ----
    Xb = tl("Xb"); Wb = tl("Wb", T + 1); cbuf = tl("cbuf", T + 1)
    negMW = tl("negMW", T + 1)
    V.memset(Wb[:, 0:1], 0.002)
    V.memset(cbuf[:, 0:1], 0.0)
    V.memset(negMW[:, 0:1], -0.001)
    sp = None
    for it in range(n_a):
        if it == 0:
            V.tensor_tensor_scan(Xb[:, :], Aa[:, :], zeros[:, :], 0.001,
                                 AL.add, AL.max)
            sp = Xb
        else:
            V.tensor_tensor(negMW[:, 1:T + 1], sp[:, :], Wb[:, 1:T + 1],
                            AL.subtract)
            V.scalar_tensor_tensor(s0[:, :], negMW[:, 0:T], 0.0, negR[:, :],
                                   AL.min, AL.max)                       # -r
            V.tensor_tensor_scan(cbuf[:, 1:T + 1], s0[:, :], s0[:, :], 0.0,
                                 AL.add, AL.bypass)                      # -cumsum r
            V.tensor_tensor_scan(Xb[:, :], Aa[:, :], cbuf[:, 0:T], 0.001,
                                 AL.add, AL.max)
            V.tensor_tensor(s1[:, :], Xb[:, :], cbuf[:, 1:T + 1], AL.subtract)
            sp = s1
        A.activation(s2[:, :], sp[:, :], AF.Copy, scale=cc(C_1CWH))
        V.tensor_tensor_scan(Wb[:, 1:T + 1], SNOW[:, :], s2[:, :], 0.002,
                             AL.add, AL.min)
    INb = tl("INb")
    V.tensor_tensor(s0[:, :], Wb[:, 0:T], Wb[:, 1:T + 1], AL.subtract)
    V.tensor_tensor(INb[:, :], s0[:, :], Pp[:, :], AL.add)

    # ---- soil: clamped Newton; reuse last iteration's e for SUZIN ----
    SMb = tl("SMb", T + 1); eb = tl("eb")
    V.memset(SMb[:, 0:1], 0.001)
    A.activation(SMb[:, 1:T + 1], ones[:, :], AF.Copy, scale=cc(C_FCH))
    INh = tl("INh", T, F16)
    g0 = tl("g0", T, F16); g1 = tl("g1", T, F16); g2 = tl("g2", T, F16)
    g3 = tl("g3", T, F16); g4 = tl("g4", T, F16); g5 = tl("g5", T, F16)
    V.tensor_copy(INh[:, :], INb[:, :])
    for it in range(n_b):
        f16 = it < 2
        Sprev = SMb[:, 0:T]
        A.activation(s0[:, :], Sprev, AF.Ln)
        if f16:
            A.activation(g1[:, :], s0[:, :], AF.Exp, scale=cc(C_BETA), bias=cc(C_BLIF))
            A.activation(g2[:, :], s0[:, :], AF.Exp, scale=cc(C_BM1), bias=cc(C_SWPB))
            A.activation(g0[:, :], g1[:, :], AF.Copy, scale=-1.0, bias=1.0)  # 1-sw
            V.tensor_tensor(g1[:, :], g0[:, :], INh[:, :], AL.mult)          # u
            V.tensor_tensor(s3[:, :], SMb[:, 0:T], g1[:, :], AL.add)         # SMa f32
            V.tensor_scalar(s4[:, :], s3[:, :], cc(C_FC), None, AL.min)      # SMmid f32
            V.tensor_scalar(g5[:, :], s4[:, :], cc(C_ILPFC), 1.0, AL.mult, AL.min)  # q
            V.tensor_tensor(g0[:, :], PEp[:, :], g5[:, :], AL.mult)          # e f16
            V.tensor_tensor(s6[:, :], s4[:, :], g0[:, :], AL.subtract)       # fval f32
            V.tensor_scalar(g3[:, :], s3[:, :], cc(C_FC), None, AL.is_lt)    # mFC
            V.tensor_tensor(g4[:, :], INh[:, :], g2[:, :], AL.mult)
            A.activation(g2[:, :], g4[:, :], AF.Copy, scale=-1.0, bias=1.0)  # 1-IN*swp
            V.tensor_scalar(g4[:, :], g5[:, :], 1.0, None, AL.is_lt)         # mEF
            V.tensor_tensor(g5[:, :], g4[:, :], PETinv[:, :], AL.mult)
            A.activation(g4[:, :], g5[:, :], AF.Copy, scale=-1.0, bias=1.0)
            V.tensor_tensor(g5[:, :], g3[:, :], g2[:, :], AL.mult)
            V.tensor_tensor(g3[:, :], g5[:, :], g4[:, :], AL.mult)
            V.tensor_scalar(g5[:, :], g3[:, :], 0.0, 1.0, AL.max, AL.min)    # a f16
            V.tensor_tensor(s0[:, :], s6[:, :], SMb[:, 1:T + 1], AL.subtract)  # rho f32
            V.tensor_tensor_scan(s1[:, :], g5[:, :], s0[:, :], 0.0, AL.mult, AL.add)
        else:
            A.activation(s1[:, :], s0[:, :], AF.Exp, scale=cc(C_BETA), bias=cc(C_BLIF))
            A.activation(s2[:, :], s0[:, :], AF.Exp, scale=cc(C_BM1), bias=cc(C_SWPB))
            A.activation(s0[:, :], s1[:, :], AF.Copy, scale=-1.0, bias=1.0)
            V.tensor_tensor(s1[:, :], s0[:, :], INb[:, :], AL.mult)
            V.tensor_tensor(s3[:, :], SMb[:, 0:T], s1[:, :], AL.add)
            V.tensor_scalar(s4[:, :], s3[:, :], cc(C_FC), None, AL.min)
            V.tensor_scalar(s5[:, :], s4[:, :], cc(C_ILPFC), 1.0, AL.mult, AL.min)
            V.tensor_tensor(eb[:, :], PEp[:, :], s5[:, :], AL.mult)
            V.tensor_tensor(s6[:, :], s4[:, :], eb[:, :], AL.subtract)
            V.tensor_scalar(s0[:, :], s3[:, :], cc(C_FC), None, AL.is_lt)
            V.tensor_tensor(s1[:, :], INb[:, :], s2[:, :], AL.mult)
            A.activation(s1[:, :], s1[:, :], AF.Copy, scale=-1.0, bias=1.0)
            V.tensor_scalar(s2[:, :], s5[:, :], 1.0, None, AL.is_lt)
            V.tensor_tensor(s2[:, :], s2[:, :], PETinv[:, :], AL.mult)
            A.activation(s2[:, :], s2[:, :], AF.Copy, scale=-1.0, bias=1.0)
            V.tensor_tensor(s3[:, :], s0[:, :], s1[:, :], AL.mult)
            V.tensor_tensor(s4[:, :], s3[:, :], s2[:, :], AL.mult)
            V.tensor_scalar(s5[:, :], s4[:, :], 0.0, 1.0, AL.max, AL.min)
            V.tensor_tensor(s0[:, :], s6[:, :], SMb[:, 1:T + 1], AL.subtract)
            V.tensor_tensor_scan(s1[:, :], s5[:, :], s0[:, :], 0.0, AL.mult, AL.add)
        V.tensor_tensor(s2[:, :], SMb[:, 1:T + 1], s1[:, :], AL.add)
        V.tensor_scalar(s3[:, :], s2[:, :], NEARZERO, None, AL.max)
        V.tensor_scalar(SMb[:, 1:T + 1], s3[:, :], cc(C_FC), None, AL.min)
    SUZIN = tl("SUZIN")
    V.tensor_tensor(s0[:, :], INb[:, :], eb[:, :], AL.subtract)
    V.tensor_tensor(s1[:, :], SMb[:, 1:T + 1], SMb[:, 0:T], AL.subtract)
    V.tensor_tensor(SUZIN[:, :], s0[:, :], s1[:, :], AL.subtract)

    # ---- SUZ regime iteration (early iterations fp16, last 2 fp32) ----
    SUZb = tl("SUZb", T + 1); SINP = tl("SINP")
    SUZh = tl("SUZh", T + 1, F16)
    SUZINh = tl("SUZINh", T, F16); SINPh = tl("SINPh", T, F16)
    h0 = tl("h0", T, F16); h1 = tl("h1", T, F16); h2 = tl("h2", T, F16)
    h3 = tl("h3", T, F16); h4 = tl("h4", T, F16); h5 = tl("h5", T, F16)
    h6 = tl("h6", T, F16)
    V.memset(SUZb[:, 0:1], 0.001)
    V.memset(SUZh[:, 0:1], 0.001)
    V.memset(SUZh[:, 1:T + 1], 0.001)
    V.tensor_scalar(SINP[:, :], SUZIN[:, :], cc(C_PCAP), None, AL.subtract)
    V.tensor_copy(SUZINh[:, :], SUZIN[:, :])
    V.tensor_copy(SINPh[:, :], SINP[:, :])
    for it in range(n_s):
        if it < n_s - 2:
            V.tensor_tensor(h0[:, :], SUZh[:, 0:T], SUZINh[:, :], AL.add)   # S1
            V.tensor_scalar(h1[:, :], h0[:, :], cc(C_PCAP), None, AL.is_gt)
            V.tensor_scalar(h2[:, :], h0[:, :], cc(C_PCUZ), None, AL.is_gt)
            V.tensor_scalar(h3[:, :], h1[:, :], cc(C_CA), None, AL.mult)
            V.tensor_scalar(h6[:, :], h2[:, :], cc(C_CB), None, AL.mult)
            V.tensor_tensor(h4[:, :], h6[:, :], h3[:, :], AL.add)           # alpha
            V.tensor_tensor(h5[:, :], h4[:, :], SINPh[:, :], AL.mult)
            V.tensor_scalar(h6[:, :], h2[:, :], cc(C_C3), None, AL.mult)
            V.tensor_tensor(h3[:, :], h5[:, :], h6[:, :], AL.add)           # beta
            V.tensor_tensor_scan(SUZh[:, 1:T + 1], h4[:, :], h3[:, :], 0.001,
                                 AL.mult, AL.add)
            continue
        prevb = SUZh if it == n_s - 2 else SUZb
        V.tensor_tensor(s0[:, :], prevb[:, 0:T], SUZIN[:, :], AL.add)      # S1
        V.tensor_scalar(s1[:, :], s0[:, :], cc(C_PCAP), None, AL.is_gt)    # m1
        V.tensor_scalar(s2[:, :], s0[:, :], cc(C_PCUZ), None, AL.is_gt)    # m2
        V.tensor_scalar(s3[:, :], s1[:, :], cc(C_CA), None, AL.mult)
        V.scalar_tensor_tensor(s4[:, :], s2[:, :], cc(C_CB), s3[:, :],
                               AL.mult, AL.add)                            # alpha
        V.tensor_tensor(s5[:, :], s4[:, :], SINP[:, :], AL.mult)
        V.scalar_tensor_tensor(s6[:, :], s2[:, :], cc(C_C3), s5[:, :],
                               AL.mult, AL.add)                            # beta
        V.tensor_tensor_scan(SUZb[:, 1:T + 1], s4[:, :], s6[:, :], 0.001,
                             AL.mult, AL.add)
    V.tensor_tensor(s0[:, :], SUZb[:, 0:T], SUZIN[:, :], AL.add)           # S1
    V.tensor_scalar(s1[:, :], s0[:, :], cc(C_PCAP), None, AL.min)          # PERC
    V.tensor_tensor(s2[:, :], s0[:, :], s1[:, :], AL.subtract)
    V.tensor_tensor(s3[:, :], s2[:, :], SUZb[:, 1:T + 1], AL.subtract)     # Q01

    # ---- SLZ ----
    A.activation(s4[:, :], ones[:, :], AF.Copy, scale=cc(C_1K2))
    A.activation(s5[:, :], s1[:, :], AF.Copy, scale=cc(C_1K2))
    V.tensor_tensor_scan(s6[:, :], s4[:, :], s5[:, :], 0.001, AL.mult, AL.add)  # SLZ
    A.activation(s0[:, :], s6[:, :], AF.Copy, scale=cc(C_KAP))             # Q2

    # ---- routing conv in fp16 ----
    QbH = tl("QbH", T + LENF - 1, F16)
    yA = tl("yA", T, F16)
    yB = tl("yB", T, F16)
    V.memset(QbH[:, 0:LENF - 1], 0.0)
    V.tensor_tensor(QbH[:, LENF - 1:T + LENF - 1], s3[:, :], s0[:, :], AL.add)  # Q
    wq = tl("wq", T, F16)
    base = LENF - 1
    V.tensor_scalar(yA[:, :], QbH[:, base:base + T], cc(C_W0), None, AL.mult)
    src, dst = yA, yB
    for k in range(1, LENF):
        V.tensor_scalar(wq[:, :], QbH[:, base - k:base - k + T],
                        cc(C_W0 + k), None, AL.mult)
        V.tensor_tensor(dst[:, :], src[:, :], wq[:, :], AL.add)
        src, dst = dst, src
    nc.gpsimd.dma_start(dout[ci], src[:, :])  # casts fp16 -> fp32


# ---------------- host orchestration ----------------
_CACHE = {}


def _get_nc(key=None):
    if key is None:
        key = (N_A, N_B, SLOT_NS)
    if key not in _CACHE:
        _CACHE[key] = build_nc(*key)
    return _CACHE[key]


def cell_layout(p, x_phy):
    """Position i (0..G_PAD-1) holds source cell cells[i]; chunk g=i//128 maps
    to core g%8, slot g//8. Hardest cells first so low slots are hard.
    Difficulty = union of two rankings: coarse-sim SUZ residual and the
    persistence heuristic (a cell is hard if either says so)."""
    G = G_FULL
    dsim = difficulty(p, x_phy)
    Pm = x_phy[:, :, 0].mean(axis=0).astype(np.float64)
    PETm = x_phy[:, :, 2].mean(axis=0).astype(np.float64)
    dcrude = (1.0 - p["parK1"]) * (Pm - 0.7 * PETm > p["parPERC"])

    def ranks(d):
        o = np.argsort(-d, kind="stable")
        r = np.empty(G, np.int64)
        r[o] = np.arange(G)
        return r

    runion = np.minimum(ranks(dsim), ranks(dcrude))
    pad = np.arange(G_PAD - G)
    rall = np.concatenate([runion, runion[pad]])
    order = np.argsort(rall, kind="stable")
    cells = np.concatenate([np.arange(G), pad])[order]
    gchunk = np.arange(G_PAD) // P128
    core_of = gchunk % N_CORES
    return cells, core_of


def kernel(x_phy: np.ndarray, parameters: np.ndarray, trace=False):
    x = np.asarray(x_phy, np.float32)
    par_last = np.asarray(parameters)[-1].astype(np.float32)
    Tn, G, _ = x.shape
    assert Tn == T and G == G_FULL
    p = host_params(par_last)
    cells, core_of = cell_layout(p, x)
    consts_all = host_consts(p)[cells]
    xg = x[:, cells, :]
    in_maps = []
    per_core = CHUNKS_PER_CORE * P128
    for c in range(N_CORES):
        idx = np.where(core_of == c)[0]
        blk = np.ascontiguousarray(np.moveaxis(xg[:, idx, :], 0, 1))  # [pc, T, 3]
        in_maps.append({
            "pp": np.ascontiguousarray(blk[:, :, 0]).reshape(CHUNKS_PER_CORE, P128, T),
            "tm": np.ascontiguousarray(blk[:, :, 1]).reshape(CHUNKS_PER_CORE, P128, T),
            "pe": np.ascontiguousarray(blk[:, :, 2]).reshape(CHUNKS_PER_CORE, P128, T),
            "cc": np.ascontiguousarray(consts_all[idx]).reshape(CHUNKS_PER_CORE, P128, NCONST),
        })
    nc = _get_nc()
    res = run_bass_kernel_spmd(nc, in_maps, list(range(N_CORES)), trace=trace)
    out = np.empty((T, G), np.float32)
    for c in range(N_CORES):
        idx = np.where(core_of == c)[0]
        ys = res.results[c]["y"].reshape(per_core, T)
        out[:, cells[idx]] = ys.T  # pad duplicates overwrite identically
    if trace:
        return out, res
    return out
